# revision 1
# baseline (speedup 1.0000x reference)
"""ALiBi causal attention on 8 TRN2 NeuronCores (Bass/Tile).

Sharding: each core computes HPC=2 heads for BOTH batches (head-parallel,
weights column-sharded).  Scores are computed transposed (S_T[k, q]) so the
ALiBi k-ramp becomes a per-partition fp32 bias applied by the ScalarEngine
exp, and the softmax-invariant q-term is folded into the score matmul as an
extra contraction row.  A ones-column appended to V makes the P@V matmul
emit softmax denominators for free.  One 8-way AllToAll redistributes
context so every core applies the full Wo to its own (batch, query-slice)
of the output.  Compute dtype bf16 (fp32 accumulation in PSUM).
"""

import math

import numpy as np
import ml_dtypes

import bass_rust
import concourse.bass as bass
import concourse.mybir as mybir
import concourse.tile as tile
from concourse.bass_utils import run_bass_kernel_spmd
from concourse.masks import make_identity

B, N, D = 2, 2048, 1024
H, HD = 16, 64
NCORES = 8
HPC = H // NCORES      # heads per core = 2
NT = N // 128          # 16 blocks of 128 along seq
QS = N // 4            # query rows owned per core = 512
QSO = N // NCORES      # query rows owned per core per batch = 256
KT = D // 128          # 8 contraction tiles for d
BF16 = mybir.dt.bfloat16
F32 = mybir.dt.float32
SHIFT = 6.0            # static upper bound of the adjusted logits


def _split_multi_waits(nc):
    """This image's walrus rejects >1 sync-wait per instruction; move extra
    waits onto single-wait NoOps spliced just before the instruction in the
    same engine stream (the engine blocks on the NoOps first)."""
    n_split = 0
    for f in nc.m.functions:
        for bb in f.blocks:
            insts = list(bb.instructions)
            new = []
            for inst in insts:
                si = getattr(inst, "sync_info", None)
                waits = list(si.on_wait) if si is not None and si.on_wait else []
                if len(waits) > 1:
                    for idx, w in enumerate(waits[1:]):
                        nop = mybir.InstNoOp(
                            name=f"{inst.name}-xw{idx}", ins=[], outs=[])
                        nop.engine = inst.engine
                        nop.sync_info = bass_rust.SyncInfo(
                            on_wait=[w], on_update=[])
                        new.append(nop)
                    si.on_wait = waits[:1]
                    n_split += 1
                new.append(inst)
            if len(new) != len(insts):
                bb.instructions = new
    return n_split


def _get_slopes(n):
    def pow2(n):
        start = 2 ** (-(2 ** (-(math.log2(n) - 3))))
        return [start * start**i for i in range(n)]

    if math.log2(n).is_integer():
        return pow2(n)
    c = 2 ** math.floor(math.log2(n))
    return pow2(c) + _get_slopes(2 * c)[0::2][: n - c]


def build_nc():
    nc = bass.Bass()

    xT = nc.declare_dram_parameter("xT", [B, 128, KT * N], BF16, isOutput=False)
    wq = nc.declare_dram_parameter("wq", [128, KT * 128], BF16, isOutput=False)
    wk = nc.declare_dram_parameter("wk", [128, KT * 128], BF16, isOutput=False)
    wv = nc.declare_dram_parameter("wv", [128, KT * 128], BF16, isOutput=False)
    wo = nc.declare_dram_parameter("wo", [128, KT * D], BF16, isOutput=False)
    qrow = nc.declare_dram_parameter("qrow", [HPC, N], BF16, isOutput=False)
    kbias = nc.declare_dram_parameter("kbias", [128, HPC * NT], F32, isOutput=False)
    maskp = nc.declare_dram_parameter("maskp", [128, 128], BF16, isOutput=False)
    y = nc.declare_dram_parameter("y", [QS, D], F32, isOutput=True)

    a2a_in = [nc.dram_tensor(f"a2a_in{b}", [NCORES, 128, QSO], BF16)
              for b in range(B)]
    a2a_out = [nc.dram_tensor(f"a2a_out{b}", [NCORES, 128, QSO], BF16)
               for b in range(B)]
    groups = [list(range(NCORES))]

    from contextlib import ExitStack

    with tile.TileContext(nc) as tc, ExitStack() as est:
        cpool = est.enter_context(tc.tile_pool(name="const", bufs=1))
        xpool = est.enter_context(tc.tile_pool(name="x", bufs=1))
        qkpool = est.enter_context(tc.tile_pool(name="qk", bufs=1))
        vpool = est.enter_context(tc.tile_pool(name="v", bufs=1))
        ppool = est.enter_context(tc.tile_pool(name="p", bufs=4))
        cnpool = est.enter_context(tc.tile_pool(name="cn", bufs=1))
        ctpool = est.enter_context(tc.tile_pool(name="ct", bufs=1))
        cfpool = est.enter_context(tc.tile_pool(name="cf", bufs=1))
        opool = est.enter_context(tc.tile_pool(name="ob", bufs=2))
        rpool = est.enter_context(tc.tile_pool(name="rc", bufs=4))
        mmps = est.enter_context(tc.tile_pool(name="mm", bufs=3, space="PSUM"))
        ops = est.enter_context(tc.tile_pool(name="o", bufs=3, space="PSUM"))
        wops = est.enter_context(tc.tile_pool(name="wo", bufs=1, space="PSUM"))

        # constants (mask comes precomputed from the host; keep the gpsimd
        # queue essentially collective-only — a collective blocks it)
        ident = cpool.tile([128, 128], F32, tag="ident", name="ident")
        nc.vector.memset(ident[:], 0.0)
        make_identity(nc, ident[:], nomemset=True)
        mask = cpool.tile([128, 128], BF16, tag="mask", name="mask")
        nc.vector.memset(mask[:], 1.0)
        nc.gpsimd.affine_select(
            out=mask[:], in_=mask[:], compare_op=mybir.AluOpType.is_ge,
            fill=0.0, base=0, pattern=[[1, 128]], channel_multiplier=-1,
        )
        zrow = cpool.tile([1, 512], BF16, tag="zrow", name="zrow")
        nc.vector.memset(zrow[:], 0.0)
        kb_sb = cpool.tile([128, HPC * NT], F32, tag="kb", name="kb_sb")
        nc.sync.dma_start(out=kb_sb[:], in_=kbias[:])
        wq_sb = cpool.tile([128, KT * 128], BF16, tag="wq", name="wq_sb")
        nc.sync.dma_start(out=wq_sb[:], in_=wq[:])
        wk_sb = cpool.tile([128, KT * 128], BF16, tag="wk", name="wk_sb")
        nc.sync.dma_start(out=wk_sb[:], in_=wk[:])
        wv_sb = cpool.tile([128, KT * 128], BF16, tag="wv", name="wv_sb")
        nc.sync.dma_start(out=wv_sb[:], in_=wv[:])
        wo_sb = cpool.tile([128, KT * D], BF16, tag="wo", name="wo_sb")
        nc.sync.dma_start(out=wo_sb[:], in_=wo[:])

        ctxT = []
        for b in range(B):
            x_t = xpool.tile([128, KT * N], BF16, tag="xt", name="x_t")
            for kt in range(KT):  # per-tile DMAs so matmuls start early
                nc.sync.dma_start(
                    out=x_t[:, kt * N:(kt + 1) * N],
                    in_=xT[b][:, kt * N:(kt + 1) * N])

            # ---- projections ----
            # qe/ke: [65, N]; rows 0..63 head data, row 64 the extra
            # contraction row (q-term for qe, ones for ke)
            qe = [qkpool.tile([65, N], BF16, tag=f"qe{b}{h}", name=f"qe{b}{h}")
                  for h in range(HPC)]
            ke = [qkpool.tile([65, N], BF16, tag=f"ke{b}{h}", name=f"ke{b}{h}")
                  for h in range(HPC)]
            for w_sb, dst in ((wq_sb, qe), (wk_sb, ke)):
                for ch in range(N // 512):
                    ps = mmps.tile([128, 512], F32, tag="mm", name="ps")
                    for kt in range(KT):
                        nc.tensor.matmul(
                            ps[:],
                            lhsT=w_sb[:, kt * 128:(kt + 1) * 128],
                            rhs=x_t[:, kt * N + ch * 512: kt * N + (ch + 1) * 512],
                            start=(kt == 0), stop=(kt == KT - 1),
                        )
                    cs = slice(ch * 512, (ch + 1) * 512)
                    nc.vector.tensor_copy(dst[0][0:64, cs], ps[0:64, :])
                    nc.vector.tensor_copy(dst[1][0:64, cs], ps[64:128, :])
            for h in range(HPC):
                nc.sync.dma_start(out=qe[h][64:65, :], in_=qrow[h:h + 1, :])
                nc.vector.memset(ke[h][64:65, :], 1.0)

            # v tiles: [128, HPC*65]; per head 64 value cols + a ones col
            v_t = [vpool.tile([128, HPC * 65], BF16, tag=f"v{b}_{nb}",
                              name=f"v{b}_{nb}")
                   for nb in range(NT)]
            for nb in range(NT):
                ps = mmps.tile([128, 128], F32, tag="mm", name="ps")
                for kt in range(KT):
                    nc.tensor.matmul(
                        ps[:],
                        lhsT=x_t[:, kt * N + nb * 128: kt * N + (nb + 1) * 128],
                        rhs=wv_sb[:, kt * 128:(kt + 1) * 128],
                        start=(kt == 0), stop=(kt == KT - 1),
                    )
                for h in range(HPC):
                    nc.vector.tensor_copy(
                        v_t[nb][:, h * 65: h * 65 + 64],
                        ps[:, h * 64:(h + 1) * 64],
                    )
                    nc.vector.memset(v_t[nb][:, h * 65 + 64: h * 65 + 65], 1.0)

            # ---- attention (per head) ----
            ctxn = [cnpool.tile([128, 128], F32, tag=f"cn{b}_{qt}",
                                name=f"cn{b}_{qt}")
                    for qt in range(NT)]
            for h in range(HPC):
                qe_ap = qe[h][:]
                ke_ap = ke[h][:]
                ot = [ops.tile([128, 6 * 65], F32, tag="o", name="ot")
                      for _ in range(3)]
                # claim + clear each O bank once (start=True clears the whole
                # bank's has_written bits, so slots must share one group)
                for t in ot:
                    nc.tensor.matmul(
                        t[:], lhsT=zrow[0:1, 0:128], rhs=zrow[0:1, 0:390],
                        start=True, stop=True,
                    )

                def oslot(qt):
                    t, s = divmod(qt, 6)
                    return ot[t][:, s * 65:(s + 1) * 65]

                for kb in range(NT):
                    q0 = kb * 128
                    for c0 in range(q0, N, 512):
                        cw = min(512, N - c0)
                        ps = mmps.tile([128, cw], F32, tag="mm", name="ps")
                        for s0 in range(0, cw, 512):
                            sw = min(512, cw - s0)
                            nc.tensor.matmul(
                                ps[:, s0:s0 + sw],
                                lhsT=ke_ap[:, kb * 128:(kb + 1) * 128],
                                rhs=qe_ap[:, c0 + s0: c0 + s0 + sw],
                            )
                        p_t = ppool.tile([128, cw], BF16, tag="p", name="p_t")
                        col = h * NT + kb
                        nc.scalar.activation(
                            p_t[:], ps[:], mybir.ActivationFunctionType.Exp,
                            bias=kb_sb[:, col:col + 1], scale=1.0,
                        )
                        if c0 == q0:  # causal mask on the diagonal block
                            nc.vector.tensor_tensor(
                                p_t[:, 0:128], p_t[:, 0:128], mask[:],
                                op=mybir.AluOpType.mult,
                            )
                        for j in range(0, cw, 128):
                            qt = (c0 + j) // 128
                            nc.tensor.matmul(
                                oslot(qt),
                                lhsT=p_t[:, j:j + 128],
                                rhs=v_t[kb][:, h * 65:(h + 1) * 65],
                                start=False, stop=(kb == qt),
                                skip_group_check=True,
                            )
                # normalize: ctx[q, :] / rowsum[q]
                for qt in range(NT):
                    osl = oslot(qt)
                    rec = rpool.tile([128, 1], F32, tag="rec", name="rec")
                    nc.vector.reciprocal(rec[:], osl[:, 64:65])
                    nc.vector.tensor_scalar(
                        out=ctxn[qt][:, h * 64:(h + 1) * 64],
                        in0=osl[:, 0:64], scalar1=rec[:], scalar2=None,
                        op0=mybir.AluOpType.mult,
                    )

            # ---- transpose ctx to [c, q] and stage the AllToAll input ----
            ct = ctpool.tile([128, N], BF16, tag=f"ct{b}", name=f"ct{b}")
            ctxT.append(ct)
            for qt in range(NT):
                trp = ops.tile([128, 128], F32, tag="o", name="trp")
                nc.tensor.transpose(trp[:], ctxn[qt][:], ident[:])
                nc.vector.tensor_copy(ct[:, qt * 128:(qt + 1) * 128], trp[:])
            nc.sync.dma_start(
                out=a2a_in[b][:].rearrange("j p q -> p j q"),
                in_=ct[:].rearrange("p (j q) -> p j q", j=NCORES),
            )

            # ---- AllToAll + output projection for this batch (batch 0's
            # collective and Wo overlap batch 1's attention) ----
            nc.gpsimd.collective_compute(
                "AllToAll", mybir.AluOpType.bypass, replica_groups=groups,
                ins=[a2a_in[b][:].opt()], outs=[a2a_out[b][:].opt()],
            )
            cf = [cfpool.tile([128, QSO], BF16, tag=f"cf{b}_{i}",
                              name=f"cf{b}_{i}")
                  for i in range(NCORES)]
            for i in range(NCORES):
                nc.sync.dma_start(out=cf[i][:], in_=a2a_out[b][i])
            for q4 in range(QSO // 128):
                ob = opool.tile([128, D], F32, tag="ob", name="ob")
                for nch in range(D // 512):
                    ps = wops.tile([128, 512], F32, tag="wo", name="wps")
                    for kt in range(KT):
                        nc.tensor.matmul(
                            ps[:],
                            lhsT=cf[kt][:, q4 * 128:(q4 + 1) * 128],
                            rhs=wo_sb[:, kt * D + nch * 512:
                                      kt * D + (nch + 1) * 512],
                            start=(kt == 0), stop=(kt == KT - 1),
                        )
                    nc.vector.tensor_copy(
                        ob[:, nch * 512:(nch + 1) * 512], ps[:])
                r0 = b * QSO + q4 * 128
                nc.sync.dma_start(out=y[r0:r0 + 128, :], in_=ob[:])

    _split_multi_waits(nc)
    return nc


_NC_CACHE = None


def _prep_inputs(x, Wq, Wk, Wv, Wo, bo):
    """Host-side sharding/layout prep. Returns in_maps for the 8 cores."""
    bf = ml_dtypes.bfloat16
    x = np.asarray(x, np.float32)
    slopes = np.array(_get_slopes(H), np.float64)

    # x transposed + tiled: [B, 128, KT*N];  xTr[b, p, kt*N+q] = x[b, q, kt*128+p]
    xTr = np.ascontiguousarray(
        x.transpose(0, 2, 1).reshape(B, KT, 128, N).transpose(0, 2, 1, 3)
        .reshape(B, 128, KT * N)
    ).astype(bf)

    def wtile(w):  # [D, m] -> [128, KT*m]
        m = w.shape[1]
        return np.ascontiguousarray(
            w.reshape(KT, 128, m).transpose(1, 0, 2).reshape(128, KT * m)
        ).astype(bf)

    # causal keep-mask in S_T layout: 1 where k(partition) <= q(free)
    pp = np.arange(128)
    maskv = (pp[:, None] <= pp[None, :]).astype(bf)

    wo_r = wtile(np.asarray(Wo, np.float32))
    in_maps = []
    for c in range(NCORES):
        hs = slice(c * HPC * HD, (c + 1) * HPC * HD)
        sl = slopes[c * HPC:(c + 1) * HPC] / 8.0
        q_idx = np.arange(N, dtype=np.float64)
        qr = (-sl[:, None] * q_idx[None, :] - SHIFT).astype(bf)
        p = np.arange(128, dtype=np.float64)
        kb = np.zeros((128, HPC * NT), np.float32)
        for h in range(HPC):
            for t in range(NT):
                kb[:, h * NT + t] = (sl[h] * (t * 128 + p)).astype(np.float32)
        in_maps.append({
            "xT": xTr,
            "wq": wtile(np.asarray(Wq, np.float32)[:, hs] / 8.0),
            "wk": wtile(np.asarray(Wk, np.float32)[:, hs]),
            "wv": wtile(np.asarray(Wv, np.float32)[:, hs]),
            "wo": wo_r,
            "qrow": qr,
            "kbias": kb,
            "maskp": maskv,
        })
    return in_maps


def _try_device_reset():
    """Best-effort NeuronCore reset via the axon client (clears collective
    state a previously killed run may have left behind)."""
    try:
        import ctypes
        import time as _time

        import jax

        jax.devices()
        lib = ctypes.CDLL("/opt/axon/libaxon_pjrt.so")
        lib.axon_reset.restype = ctypes.c_int64
        lib.axon_reset()
        _time.sleep(5)
    except Exception:
        pass


def kernel(x, Wq, Wk, Wv, Wo, bo):
    global _NC_CACHE
    if _NC_CACHE is None:
        _NC_CACHE = build_nc()
    nc = _NC_CACHE
    in_maps = _prep_inputs(x, Wq, Wk, Wv, Wo, bo)
    try:
        res = run_bass_kernel_spmd(nc, in_maps, list(range(NCORES)))
    except Exception:
        _try_device_reset()
        res = run_bass_kernel_spmd(nc, in_maps, list(range(NCORES)))
    out = np.empty((B, N, D), np.float32)
    for c in range(NCORES):
        for b in range(B):
            out[b, c * QSO:(c + 1) * QSO, :] = \
                res.results[c]["y"][b * QSO:(b + 1) * QSO]
    out += np.asarray(bo, np.float32)[None, None, :]
    return out



# revision 7
# speedup vs baseline: 1.1369x; 1.1369x over previous
"""ALiBi causal attention on 8 TRN2 NeuronCores (Bass/Tile).

Sharding: each core computes HPC=2 heads for BOTH batches (head-parallel,
weights column-sharded).  Scores are computed transposed (S_T[k, q]) so the
ALiBi k-ramp becomes a per-partition fp32 bias applied by the ScalarEngine
exp, and the softmax-invariant q-term is folded into the score matmul as an
extra contraction row.  P@V is computed V-stationary: ctx is accumulated
directly transposed (ctx[c, q]) in a 4-bank PSUM accumulator with wide
moving-operand streams, and a ones-column in V emits softmax denominators
for free.  Normalization broadcasts the reciprocal denominator row across
partitions with a rank-1 PE matmul, then one DVE multiply per 512-chunk.
Both batches' compute is issued before either batch's output projection so
the AllToAll collectives overlap attention instead of stalling the in-order
PE queue.  Compute dtype bf16 (fp32 accumulation in PSUM).
"""

import math

import numpy as np
import ml_dtypes

import bass_rust
import concourse.bass as bass
import concourse.mybir as mybir
import concourse.tile as tile
from concourse.bass_utils import run_bass_kernel_spmd

B, N, D = 2, 2048, 1024
H, HD = 16, 64
NCORES = 8
HPC = H // NCORES      # heads per core = 2
NT = N // 128          # 16 blocks of 128 along seq
NCH = N // 512         # 4 column chunks of 512 along seq
QS = N // 4            # query rows owned per core = 512
QSO = N // NCORES      # query rows owned per core per batch = 256
KT = D // 128          # 8 contraction tiles for d
BF16 = mybir.dt.bfloat16
F32 = mybir.dt.float32
SHIFT = 6.0            # static upper bound of the adjusted logits


def _split_multi_waits(nc):
    """This image's walrus rejects >1 sync-wait per instruction; move extra
    waits onto single-wait NoOps spliced just before the instruction in the
    same engine stream (the engine blocks on the NoOps first)."""
    n_split = 0
    for f in nc.m.functions:
        for bb in f.blocks:
            insts = list(bb.instructions)
            new = []
            for inst in insts:
                si = getattr(inst, "sync_info", None)
                waits = list(si.on_wait) if si is not None and si.on_wait else []
                if len(waits) > 1:
                    for idx, w in enumerate(waits[1:]):
                        nop = mybir.InstNoOp(
                            name=f"{inst.name}-xw{idx}", ins=[], outs=[])
                        nop.engine = inst.engine
                        nop.sync_info = bass_rust.SyncInfo(
                            on_wait=[w], on_update=[])
                        new.append(nop)
                    si.on_wait = waits[:1]
                    n_split += 1
                new.append(inst)
            if len(new) != len(insts):
                bb.instructions = new
    return n_split


def _get_slopes(n):
    def pow2(n):
        start = 2 ** (-(2 ** (-(math.log2(n) - 3))))
        return [start * start**i for i in range(n)]

    if math.log2(n).is_integer():
        return pow2(n)
    c = 2 ** math.floor(math.log2(n))
    return pow2(c) + _get_slopes(2 * c)[0::2][: n - c]


def _chunks_for_kb(kb):
    """512-aligned chunk list [(c0, cw), ...] covering [kb*128, N)."""
    q0 = kb * 128
    out = []
    c0 = q0
    while c0 < N:
        end = min((c0 // 512 + 1) * 512, N)
        out.append((c0, end - c0))
        c0 = end
    return out


def build_nc():
    nc = bass.Bass()

    xT = nc.declare_dram_parameter("xT", [B, 128, NCH * KT * 512], BF16,
                                   isOutput=False)
    wq = nc.declare_dram_parameter("wq", [128, KT * 128], BF16, isOutput=False)
    wk = nc.declare_dram_parameter("wk", [128, KT * 128], BF16, isOutput=False)
    wv = nc.declare_dram_parameter("wv", [128, KT * 128], BF16, isOutput=False)
    wo = nc.declare_dram_parameter("wo", [128, KT * D], BF16, isOutput=False)
    qrow = nc.declare_dram_parameter("qrow", [HPC, N], BF16, isOutput=False)
    kbias = nc.declare_dram_parameter("kbias", [128, HPC * NT], F32,
                                      isOutput=False)
    y = nc.declare_dram_parameter("y", [QS, D], F32, isOutput=True)

    a2a_in = [nc.dram_tensor(f"a2a_in{b}", [NCORES, 128, QSO], BF16)
              for b in range(B)]
    a2a_out = [nc.dram_tensor(f"a2a_out{b}", [NCORES, 128, QSO], BF16)
               for b in range(B)]
    groups = [list(range(NCORES))]

    from contextlib import ExitStack

    with tile.TileContext(nc) as tc, ExitStack() as est:
        cpool = est.enter_context(tc.tile_pool(name="const", bufs=1))
        xpool = est.enter_context(tc.tile_pool(name="x", bufs=1))
        qkpool = est.enter_context(tc.tile_pool(name="qk", bufs=1))
        vpool = est.enter_context(tc.tile_pool(name="v", bufs=1))
        ppool = est.enter_context(tc.tile_pool(name="p", bufs=4))
        rpool = est.enter_context(tc.tile_pool(name="rc", bufs=2))
        ctpool = est.enter_context(tc.tile_pool(name="ct", bufs=1))
        cfpool = est.enter_context(tc.tile_pool(name="cf", bufs=1))
        opool = est.enter_context(tc.tile_pool(name="ob", bufs=2))
        mmps = est.enter_context(tc.tile_pool(name="mm", bufs=4, space="PSUM"))
        ctxps = est.enter_context(tc.tile_pool(name="cx", bufs=1, space="PSUM"))

        # ---- constants ----
        mask = cpool.tile([128, 128], BF16, tag="mask", name="mask")
        nc.vector.memset(mask[:], 1.0)
        nc.gpsimd.affine_select(
            out=mask[:], in_=mask[:], compare_op=mybir.AluOpType.is_ge,
            fill=0.0, base=0, pattern=[[1, 128]], channel_multiplier=-1,
        )
        ones64 = cpool.tile([1, 64], BF16, tag="ones64", name="ones64")
        nc.vector.memset(ones64[:], 1.0)
        kb_sb = cpool.tile([128, HPC * NT], F32, tag="kb", name="kb_sb")
        nc.sync.dma_start(out=kb_sb[:], in_=kbias[:])
        wq_sb = cpool.tile([128, KT * 128], BF16, tag="wq", name="wq_sb")
        nc.sync.dma_start(out=wq_sb[:], in_=wq[:])
        wk_sb = cpool.tile([128, KT * 128], BF16, tag="wk", name="wk_sb")
        nc.sync.dma_start(out=wk_sb[:], in_=wk[:])
        wv_sb = cpool.tile([128, KT * 128], BF16, tag="wv", name="wv_sb")
        nc.sync.dma_start(out=wv_sb[:], in_=wv[:])
        wo_sb = cpool.tile([128, KT * D], BF16, tag="wo", name="wo_sb")
        nc.sync.dma_start(out=wo_sb[:], in_=wo[:])

        # x tiles, chunk-contiguous: cols [ch*KT*512 + kt*512 + j]
        x_t = []
        for b in range(B):
            xt = xpool.tile([128, NCH * KT * 512], BF16, tag=f"xt{b}",
                            name=f"x_t{b}")
            x_t.append(xt)
            for ch in range(NCH):
                cs = slice(ch * KT * 512, (ch + 1) * KT * 512)
                nc.sync.dma_start(out=xt[:, cs], in_=xT[b][:, cs])

        def xcol(b, ch, kt):
            base = ch * KT * 512 + kt * 512
            return x_t[b][:, base:base + 512]

        ct = [ctpool.tile([128, N], BF16, tag=f"ct{b}", name=f"ct{b}")
              for b in range(B)]
        cf = [[cfpool.tile([128, QSO], BF16, tag=f"cf{b}_{i}",
                           name=f"cf{b}_{i}")
               for i in range(NCORES)] for b in range(B)]

        def compute_batch(b):
            # ---- Q/K projections: qe/ke [65, N], rows 0..63 head data,
            # row 64 the extra contraction row (q-term / ones) ----
            qe = [qkpool.tile([65, N], BF16, tag=f"qe{b}{h}", name=f"qe{b}{h}")
                  for h in range(HPC)]
            ke = [qkpool.tile([65, N], BF16, tag=f"ke{b}{h}", name=f"ke{b}{h}")
                  for h in range(HPC)]
            for h in range(HPC):
                nc.sync.dma_start(out=qe[h][64:65, :], in_=qrow[h:h + 1, :])
                nc.vector.memset(ke[h][64:65, :], 1.0)
            for w_sb, dst, veng in ((wq_sb, qe, True), (wk_sb, ke, False)):
                for ch in range(NCH):
                    ps = mmps.tile([128, 512], F32, tag="mm", name="ps")
                    for kt in range(KT):
                        nc.tensor.matmul(
                            ps[:],
                            lhsT=w_sb[:, kt * 128:(kt + 1) * 128],
                            rhs=xcol(b, ch, kt),
                            start=(kt == 0), stop=(kt == KT - 1),
                        )
                    cs = slice(ch * 512, (ch + 1) * 512)
                    if veng:  # q copies on DVE, k copies on ACT (balance)
                        nc.vector.tensor_copy(dst[0][0:64, cs], ps[0:64, :])
                        nc.vector.tensor_copy(dst[1][0:64, cs], ps[64:128, :])
                    else:
                        nc.scalar.copy(dst[0][0:64, cs], ps[0:64, :])
                        nc.scalar.copy(dst[1][0:64, cs], ps[64:128, :])

            # ---- V: v_t[nb] [128, 130]; per head 64 value cols + ones col
            # (cols h*65..h*65+64), built with one strided DVE copy ----
            v_t = [vpool.tile([128, HPC * 65], BF16, tag=f"v{b}_{nb}",
                              name=f"v{b}_{nb}")
                   for nb in range(NT)]
            for nb in range(NT):
                ps = mmps.tile([128, 512], F32, tag="mm", name="vps")
                for kt in range(KT):
                    nc.tensor.matmul(
                        ps[:, 0:128],
                        lhsT=xcol(b, nb // 4, kt)[:, (nb % 4) * 128:
                                                  (nb % 4) * 128 + 128],
                        rhs=wv_sb[:, kt * 128:(kt + 1) * 128],
                        start=(kt == 0), stop=(kt == KT - 1),
                    )
                vr = v_t[nb][:].rearrange("p (g c) -> p g c", g=HPC)
                sr = ps[:, 0:128].rearrange("p (g c) -> p g c", g=HPC)
                nc.vector.tensor_copy(vr[:, :, 0:64], sr[:])
                nc.vector.memset(vr[:, :, 64:65], 1.0)

            # ---- attention (per head): ctx accumulated transposed ----
            for h in range(HPC):
                ctx = ctxps.tile([128, N], F32, tag="ctx", name="ctx")
                vsl = slice(h * 65, (h + 1) * 65)
                for kb in range(NT):
                    q0 = kb * 128
                    chunks = _chunks_for_kb(kb)
                    pts = []
                    for ci, (c0, cw) in enumerate(chunks):
                        ps = mmps.tile([128, 512], F32, tag="mm", name="sps")
                        nc.tensor.matmul(
                            ps[:, 0:cw],
                            lhsT=ke[h][:, q0:q0 + 128],
                            rhs=qe[h][:, c0:c0 + cw],
                            start=True, stop=True,
                        )
                        p_t = ppool.tile([128, 512], BF16, tag="p", name="p_t")
                        pts.append(p_t)
                        col = h * NT + kb
                        nc.scalar.activation(
                            p_t[:, 0:cw], ps[:, 0:cw],
                            mybir.ActivationFunctionType.Exp,
                            bias=kb_sb[:, col:col + 1], scale=1.0,
                        )
                        if ci == 0:  # causal mask on the diagonal block
                            nc.vector.tensor_tensor(
                                p_t[:, 0:128], p_t[:, 0:128], mask[:],
                                op=mybir.AluOpType.mult,
                            )
                    # P@V, V-stationary; diagonal sub-block last (its mask
                    # dependency hides behind the other chunks), except at
                    # kb==0 where it must carry the bank-clearing start.
                    c00, cw0 = chunks[0]
                    if kb == 0:
                        for ci, (c0, cw) in enumerate(chunks):
                            nc.tensor.matmul(
                                ctx[0:65, c0:c0 + cw],
                                lhsT=v_t[kb][:, vsl], rhs=pts[ci][:, 0:cw],
                                start=True, stop=(cw == 128),
                                skip_group_check=True,
                            )
                    else:
                        if cw0 > 128:  # first chunk minus diagonal block
                            nc.tensor.matmul(
                                ctx[0:65, c00 + 128:c00 + cw0],
                                lhsT=v_t[kb][:, vsl],
                                rhs=pts[0][:, 128:cw0],
                                start=False, stop=False,
                                skip_group_check=True,
                            )
                        for ci, (c0, cw) in enumerate(chunks[1:], 1):
                            nc.tensor.matmul(
                                ctx[0:65, c0:c0 + cw],
                                lhsT=v_t[kb][:, vsl], rhs=pts[ci][:, 0:cw],
                                start=False, stop=False,
                                skip_group_check=True,
                            )
                        nc.tensor.matmul(  # diagonal block: final write
                            ctx[0:65, c00:c00 + 128],
                            lhsT=v_t[kb][:, vsl], rhs=pts[0][:, 0:128],
                            start=False, stop=True,
                            skip_group_check=True,
                        )
                # normalize: ct[h rows, q] = ctx[0:64, q] * (1/ctx[64, q]).
                # Denominator row -> SBUF bf16 (ScalarE), broadcast across
                # partitions by a rank-1 bf16 matmul, reciprocal on the
                # broadcast (DVE, PSUM->SBUF), then one multiply per chunk
                # (DVE reads at most one PSUM operand per instruction).
                den = rpool.tile([1, N], BF16, tag="den", name="den")
                nc.scalar.copy(den[:], ctx[64:65, :])
                for c0 in range(0, N, 512):
                    bc = mmps.tile([128, 512], F32, tag="mm", name="bc")
                    nc.tensor.matmul(
                        bc[0:64, :], lhsT=ones64[:],
                        rhs=den[:, c0:c0 + 512], start=True, stop=True,
                    )
                    bcr = rpool.tile([64, 512], F32, tag="bcr", name="bcr")
                    nc.vector.reciprocal(bcr[:], bc[0:64, :])
                    nc.vector.tensor_tensor(
                        ct[b][h * 64:(h + 1) * 64, c0:c0 + 512],
                        ctx[0:64, c0:c0 + 512], bcr[:],
                        op=mybir.AluOpType.mult,
                    )
            # ---- stage the AllToAll input ----
            nc.sync.dma_start(
                out=a2a_in[b][:].rearrange("j p q -> p j q"),
                in_=ct[b][:].rearrange("p (j q) -> p j q", j=NCORES),
            )

        def collective(b):
            nc.gpsimd.collective_compute(
                "AllToAll", mybir.AluOpType.bypass, replica_groups=groups,
                ins=[a2a_in[b][:].opt()], outs=[a2a_out[b][:].opt()],
            )
            for i in range(NCORES):
                nc.sync.dma_start(out=cf[b][i][:], in_=a2a_out[b][i])

        def out_proj(b):
            for q4 in range(QSO // 128):
                ob = opool.tile([128, D], F32, tag="ob", name="ob")
                for nch in range(D // 512):
                    ps = mmps.tile([128, 512], F32, tag="mm", name="wps")
                    for kt in range(KT):
                        nc.tensor.matmul(
                            ps[:],
                            lhsT=cf[b][kt][:, q4 * 128:(q4 + 1) * 128],
                            rhs=wo_sb[:, kt * D + nch * 512:
                                      kt * D + (nch + 1) * 512],
                            start=(kt == 0), stop=(kt == KT - 1),
                        )
                    nc.vector.tensor_copy(
                        ob[:, nch * 512:(nch + 1) * 512], ps[:])
                r0 = b * QSO + q4 * 128
                nc.sync.dma_start(out=y[r0:r0 + 128, :], in_=ob[:])

        # Both batches' compute first; output projections last so the
        # in-order PE queue never waits on a collective.
        compute_batch(0)
        collective(0)
        compute_batch(1)
        collective(1)
        out_proj(0)
        out_proj(1)

    _split_multi_waits(nc)
    return nc


_NC_CACHE = None


def _prep_inputs(x, Wq, Wk, Wv, Wo, bo):
    """Host-side sharding/layout prep. Returns in_maps for the 8 cores."""
    bf = ml_dtypes.bfloat16
    x = np.asarray(x, np.float32)
    slopes = np.array(_get_slopes(H), np.float64)

    # x transposed, chunk-contiguous: xTr[b, p, ch*KT*512 + kt*512 + j]
    #   = x[b, ch*512 + j, kt*128 + p]
    xTr = np.ascontiguousarray(
        x.transpose(0, 2, 1)                     # [B, D, N]
        .reshape(B, KT, 128, NCH, 512)
        .transpose(0, 2, 3, 1, 4)                # [B, 128, NCH, KT, 512]
        .reshape(B, 128, NCH * KT * 512)
    ).astype(bf)

    def wtile(w):  # [D, m] -> [128, KT*m]
        m = w.shape[1]
        return np.ascontiguousarray(
            w.reshape(KT, 128, m).transpose(1, 0, 2).reshape(128, KT * m)
        ).astype(bf)

    wo_r = wtile(np.asarray(Wo, np.float32))
    in_maps = []
    for c in range(NCORES):
        hs = slice(c * HPC * HD, (c + 1) * HPC * HD)
        sl = slopes[c * HPC:(c + 1) * HPC] / 8.0
        q_idx = np.arange(N, dtype=np.float64)
        qr = (-sl[:, None] * q_idx[None, :] - SHIFT).astype(bf)
        p = np.arange(128, dtype=np.float64)
        kb = np.zeros((128, HPC * NT), np.float32)
        for h in range(HPC):
            for t in range(NT):
                kb[:, h * NT + t] = (sl[h] * (t * 128 + p)).astype(np.float32)
        in_maps.append({
            "xT": xTr,
            "wq": wtile(np.asarray(Wq, np.float32)[:, hs] / 8.0),
            "wk": wtile(np.asarray(Wk, np.float32)[:, hs]),
            "wv": wtile(np.asarray(Wv, np.float32)[:, hs]),
            "wo": wo_r,
            "qrow": qr,
            "kbias": kb,
        })
    return in_maps


def _try_device_reset():
    """Best-effort NeuronCore reset via the axon client (clears collective
    state a previously killed run may have left behind)."""
    try:
        import ctypes
        import time as _time

        import jax

        jax.devices()
        lib = ctypes.CDLL("/opt/axon/libaxon_pjrt.so")
        lib.axon_reset.restype = ctypes.c_int64
        lib.axon_reset()
        _time.sleep(5)
    except Exception:
        pass


def kernel(x, Wq, Wk, Wv, Wo, bo):
    global _NC_CACHE
    if _NC_CACHE is None:
        _NC_CACHE = build_nc()
    nc = _NC_CACHE
    in_maps = _prep_inputs(x, Wq, Wk, Wv, Wo, bo)
    try:
        res = run_bass_kernel_spmd(nc, in_maps, list(range(NCORES)))
    except Exception:
        _try_device_reset()
        res = run_bass_kernel_spmd(nc, in_maps, list(range(NCORES)))
    out = np.empty((B, N, D), np.float32)
    for c in range(NCORES):
        for b in range(B):
            out[b, c * QSO:(c + 1) * QSO, :] = \
                res.results[c]["y"][b * QSO:(b + 1) * QSO]
    out += np.asarray(bo, np.float32)[None, None, :]
    return out
